# revision 6
# baseline (speedup 1.0000x reference)
"""Causal self-attention (B=2, S=2048, D=1024, H=16) on 8 TRN2 NeuronCores.

Zero-communication sharding: core c -> (batch b=c//4, query-set qs=(c%4)//2,
head-half m=c%2). The two cores sharing (b, qs) split the 16 heads; each
computes K/V for its 8 heads over the FULL sequence (no replication
anywhere), attention for the query-set's 1024 rows, and a partial output
projection over its 512 head-dims. Host sums the two partials + bias.

Causal handling: per batch, query chunks of 256 rows; query-set 0 gets
chunks [7,5,2,0] (group-ordered), set 1 gets [6,4,3,1]. Attention per head
runs 4 static slot-groups with T=(16,12,8,4) kv-block slots; slot s of
group g multiplies kv block s (128 tokens) against the group's 256 q
columns. The last 4 slots of each group carry a per-core mask (causal
diagonals / -1e9 padding); earlier slots are always fully visible.

V projection runs first, then per head-pair: Q proj, K proj, attention for
its two heads -- the scalar-engine exp backlog of a pair drains while the
PE runs the next pair's projections.

All matmul operands bf16 (full PE rate, all N>=256), f32 PSUM accumulation.
Denominators via an appended ones-column in V (row 64 of the AV psum).
"""

import os
import numpy as np
import ml_dtypes

import concourse.bass as bass
import concourse.mybir as mybir
import concourse.tile as tile
from concourse import bacc
from concourse.bass_utils import run_bass_kernel_spmd

F32 = mybir.dt.float32
BF16 = mybir.dt.bfloat16
AF = mybir.ActivationFunctionType
ALU = mybir.AluOpType

B, S, D, H, HD = 2, 2048, 1024, 16, 64
NKC = D // 128          # 8 contraction chunks of the model dim
NHP = 4                 # head-pairs per core (8 heads)
QL = 1024               # query rows per core
GW = 256                # query-group width
NEG = -1.0e9
SCALE = 1.0 / np.sqrt(HD)
TG = [16, 12, 8, 4]     # kv-block slots per group
SUBT = {g: [4] * (t // 4) for g, t in enumerate(TG)}   # psum sub-tiles of 4 slots

QSETS = [
    [7, 5, 2, 0],   # needs 16,12,6,2  <= TG
    [6, 4, 3, 1],   # needs 14,10,8,4  <= TG
]

_CACHED = {}


def build_nc():
    nc = bacc.Bacc("TRN2", target_bir_lowering=False, debug=False, num_devices=8)

    xt = nc.dram_tensor("xt", [D, S], BF16, kind="ExternalInput").ap()
    xq = nc.dram_tensor("xq", [D, QL], BF16, kind="ExternalInput").ap()
    wa = nc.dram_tensor("wa", [D, 1536], BF16, kind="ExternalInput").ap()
    wp = nc.dram_tensor("wp", [512, D], BF16, kind="ExternalInput").ap()
    bq = nc.dram_tensor("bq", [512], F32, kind="ExternalInput").ap()
    bk = nc.dram_tensor("bk", [512], F32, kind="ExternalInput").ap()
    bav = nc.dram_tensor("bav", [1, 512], BF16, kind="ExternalInput").ap()
    maskt = nc.dram_tensor("maskt", [128, 4 * 1024], BF16, kind="ExternalInput").ap()
    ones1 = nc.dram_tensor("ones1", [1, 128], BF16, kind="ExternalInput").ap()
    ident = nc.dram_tensor("ident", [128, 128], BF16, kind="ExternalInput").ap()
    out = nc.dram_tensor("out", [QL, D], F32, kind="ExternalOutput").ap()

    with tile.TileContext(nc) as tc:
        _body(nc, tc, xt, xq, wa, wp, bq, bk, bav, maskt, ones1, ident, out)
    nc.compile()
    return nc


def _body(nc, tc, xt, xq, wa, wp, bq, bk, bav, maskt, ones1, ident, out):
    with (
        tc.tile_pool(name="const", bufs=1) as const_p,
        tc.tile_pool(name="qt", bufs=1) as qt_p,
        tc.tile_pool(name="kt", bufs=1) as kt_p,
        tc.tile_pool(name="vv", bufs=1) as v_p,
        tc.tile_pool(name="yt", bufs=1) as yt_p,
    ):
        # ---------- constants ----------
        ones_s = const_p.tile([1, 128], BF16)
        nc.sync.dma_start(ones_s[:], ones1)
        ident_s = const_p.tile([128, 128], BF16)
        nc.sync.dma_start(ident_s[:], ident)
        bav_s = const_p.tile([1, 512], BF16)
        nc.sync.dma_start(bav_s[:], bav)
        mask_s = const_p.tile([128, 4 * 1024], BF16)
        bq_s = [const_p.tile([128, 1], F32, tag=f"bq{i}", name=f"bq{i}")
                for i in range(NHP)]
        bk_s = [const_p.tile([128, 1], F32, tag=f"bk{i}", name=f"bk{i}")
                for i in range(NHP)]

        bav_bc = const_p.tile([128, 512], BF16)

        # persistent SBUF: Q^T, K^T, V, y^T
        qt_s = [qt_p.tile([128, QL], BF16, tag=f"qt{hp}", name=f"qt{hp}")
                for hp in range(NHP)]
        kt_s = [kt_p.tile([128, S], BF16, tag=f"kt{hp}", name=f"kt{hp}")
                for hp in range(NHP)]
        # V: 16 kv-blocks x [128, 8 heads * 65]; ones col at slot 64
        v_s = v_p.tile([128, 16 * 520], BF16)
        ytr_s = [yt_p.tile([128, QL], BF16, tag=f"ytr{hp}", name=f"ytr{hp}")
                 for hp in range(NHP)]

        # ---------- phases 1+2 interleaved ----------
        # V proj first (all 16 kv blocks), then per head-pair: Q proj, K proj,
        # and attention for its two heads. The scalar-engine exp backlog of a
        # head-pair drains while the PE runs the next pair's projections.
        TCH = 512
        NTC = S // TCH
        with (
            tc.tile_pool(name="waw", bufs=1) as wa_p,
            tc.tile_pool(name="xqp", bufs=1) as xq_p,
            tc.tile_pool(name="xtc", bufs=1) as xtc_p,
            tc.tile_pool(name="pt", bufs=5) as p_p,
            tc.tile_pool(name="den", bufs=2) as den_p,
            tc.tile_pool(name="psum", bufs=2, space="PSUM") as psum,
            tc.tile_pool(name="spsum", bufs=2, space="PSUM") as spsum,
            tc.tile_pool(name="opsum", bufs=1, space="PSUM") as opsum,
        ):
            t = psum.tile([128, 512], F32, tag="ps", name="bavbc")
            nc.tensor.matmul(t[:], ones_s[:], bav_s[:], start=True, stop=True)
            nc.vector.tensor_copy(bav_bc[:], t[:])

            # batched loads, ordered by first use: Wv + x chunks first (V proj
            # starts after ~2MB lands), then Wq/Wk, xq, biases, masks, wp
            wav_s = wa_p.tile([128, NKC * 512], BF16, tag="wav")
            nc.sync.dma_start(
                wav_s[:].rearrange("p (kc n) -> p kc n", n=512),
                wa[:, 1024:1536].rearrange("(kc p) n -> p kc n", p=128))
            xt_c = [xtc_p.tile([128, NKC * TCH], BF16, tag=f"xt{tcn}",
                               name=f"xtc{tcn}") for tcn in range(NTC)]
            for tcn in range(NTC):
                nc.sync.dma_start(
                    xt_c[tcn][:].rearrange("p (kc n) -> p kc n", n=TCH),
                    xt[:, tcn * TCH:(tcn + 1) * TCH].rearrange(
                        "(kc p) n -> p kc n", p=128))
            waqk_s = wa_p.tile([128, NKC * 1024], BF16, tag="waqk")
            nc.sync.dma_start(
                waqk_s[:].rearrange("p (kc n) -> p kc n", n=1024),
                wa[:, 0:1024].rearrange("(kc p) n -> p kc n", p=128))
            xq_s = xq_p.tile([128, NKC * QL], BF16)
            nc.sync.dma_start(
                xq_s[:].rearrange("p (kc n) -> p kc n", n=QL),
                xq.rearrange("(kc p) n -> p kc n", p=128))
            for hp in range(NHP):
                nc.sync.dma_start(
                    bq_s[hp][:],
                    bq[hp * 128:(hp + 1) * 128].rearrange("(p o) -> p o", o=1))
                nc.sync.dma_start(
                    bk_s[hp][:],
                    bk[hp * 128:(hp + 1) * 128].rearrange("(p o) -> p o", o=1))
            nc.sync.dma_start(mask_s[:], maskt)

            def wa_c(kc, lo, hi):
                if lo >= 1024:
                    return wav_s[:, kc * 512 + lo - 1024:kc * 512 + hi - 1024]
                return waqk_s[:, kc * 1024 + lo:kc * 1024 + hi]

            wp_s = wa_p.tile([128, NHP * D], BF16, tag="wp")
            nc.sync.dma_start(
                wp_s[:].rearrange("p (hp n) -> p hp n", n=D),
                wp.rearrange("(hp p) n -> p hp n", p=128))
            # out-projection accumulator in SBUF: 8 row-chunks x [128, 1024]
            acc_s = yt_p.tile([128, 8 * D], F32, tag="acc")

            # V (all blocks, all 8 heads)
            for tcn in range(NTC):
                for vt in range(TCH // 128):
                    blk = tcn * (TCH // 128) + vt
                    ps = psum.tile([128, 512], F32, tag="ps", name=f"v{blk}")
                    for kc in range(NKC):
                        nc.tensor.matmul(
                            ps[:],
                            xt_c[tcn][:, kc * TCH + vt * 128:kc * TCH + (vt + 1) * 128],
                            wa_c(kc, 1024, 1536),
                            start=(kc == 0), stop=(kc == NKC - 1))
                    v3 = v_s[:, blk * 520:(blk + 1) * 520].rearrange(
                        "p (h e) -> p h e", e=65)
                    nc.vector.tensor_tensor(
                        v3[:, :, 0:64],
                        ps[:].rearrange("p (h e) -> p h e", e=64),
                        bav_bc[:].rearrange("p (h e) -> p h e", e=64),
                        ALU.add)
                    nc.vector.memset(v3[:, :, 64:65], 1.0)

            # projection chain emitters: two Q halves + four K chunks per pair
            def q_chain(hp, half):
                ps = psum.tile([128, 512], F32, tag="ps", name=f"q{hp}_{half}")
                for kc in range(NKC):
                    nc.tensor.matmul(
                        ps[:], wa_c(kc, hp * 128, (hp + 1) * 128),
                        xq_s[:, kc * QL + half * 512:kc * QL + (half + 1) * 512],
                        start=(kc == 0), stop=(kc == NKC - 1))
                nc.vector.tensor_scalar(
                    qt_s[hp][:, half * 512:(half + 1) * 512], ps[:],
                    SCALE, bq_s[hp][:], ALU.mult, ALU.add)

            def k_chain(hp, tcn):
                ps = psum.tile([128, TCH], F32, tag="ps", name=f"k{hp}_{tcn}")
                for kc in range(NKC):
                    nc.tensor.matmul(
                        ps[:], wa_c(kc, 512 + hp * 128, 512 + (hp + 1) * 128),
                        xt_c[tcn][:, kc * TCH:(kc + 1) * TCH],
                        start=(kc == 0), stop=(kc == NKC - 1))
                nc.vector.tensor_scalar(
                    kt_s[hp][:, tcn * TCH:(tcn + 1) * TCH], ps[:],
                    bk_s[hp][:], None, ALU.add)

            def pair_chains(hp):
                out = []
                for half in range(2):
                    out.append(lambda hp=hp, half=half: q_chain(hp, half))
                for tcn in range(NTC):
                    out.append(lambda hp=hp, tcn=tcn: k_chain(hp, tcn))
                return out

            # pair 0's projections up front
            for fn in pair_chains(0):
                fn()

            for hp in range(NHP):
                # chains of the NEXT pair, drip-fed between attention subtiles
                # so the PE has independent work while exp results are pending
                chains = pair_chains(hp + 1) if hp + 1 < NHP else []
                ci = 0
                sub_i = 0

                # attention for this pair's two heads
                for h in (2 * hp, 2 * hp + 1):
                    hh = h % 2
                    opL = opsum.tile([65, 512], F32, tag="opL", name=f"opL{h}")
                    opR = opsum.tile([65, 512], F32, tag="opR", name=f"opR{h}")
                    for g in range(4):
                        op_t = opL if g < 2 else opR
                        oc = (g % 2) * GW
                        s0 = 0
                        for si, ns in enumerate(SUBT[g]):
                            st = spsum.tile([128, ns * GW], F32, tag="sp",
                                            name=f"s{h}_{g}_{si}")
                            tail = (si == len(SUBT[g]) - 1)
                            if tail:
                                # masked tail: open each psum bank with the mask
                                # (I^T @ mask zeroes + writes it), then the S
                                # matmuls accumulate on top
                                for mb in range(ns // 2):
                                    nc.tensor.matmul(
                                        st[:, mb * 512:(mb + 1) * 512], ident_s[:],
                                        mask_s[:, g * 1024 + mb * 512:
                                               g * 1024 + (mb + 1) * 512],
                                        start=True, stop=False)
                            for sl in range(ns):
                                s = s0 + sl
                                nc.tensor.matmul(
                                    st[:, sl * GW:(sl + 1) * GW],
                                    kt_s[hp][hh * 64:(hh + 1) * 64,
                                             s * 128:(s + 1) * 128],
                                    qt_s[hp][hh * 64:(hh + 1) * 64,
                                             g * GW:(g + 1) * GW],
                                    start=not tail, stop=(not tail) or sl % 2 == 1)
                            pt = p_p.tile([128, ns * GW], BF16, tag="p",
                                          name=f"pt{h}_{g}_{si}")
                            nc.scalar.activation(pt[:], st[:], AF.Exp)
                            # drip one projection chain every ~3rd subtile
                            if ci < len(chains) and sub_i % 3 == 2:
                                chains[ci]()
                                ci += 1
                            sub_i += 1
                            for sl in range(ns):
                                s = s0 + sl
                                nc.tensor.matmul(
                                    op_t[:, oc:oc + GW],
                                    v_s[:, s * 520 + h * 65:s * 520 + (h + 1) * 65],
                                    pt[:, sl * GW:(sl + 1) * GW],
                                    start=(s == 0), stop=(s == TG[g] - 1))
                            s0 += ns
                    # normalize: row 64 = denominators
                    den_h = den_p.tile([1, QL], F32, tag="den", name=f"den{h}")
                    nc.vector.tensor_copy(den_h[:, 0:512], opL[64:65, :])
                    nc.vector.tensor_copy(den_h[:, 512:1024], opR[64:65, :])
                    rec_h = den_p.tile([1, QL], BF16, tag="rec", name=f"rec{h}")
                    with nc.allow_low_precision(reason="bf16 reciprocal of denom"):
                        nc.vector.reciprocal(rec_h[:], den_h[:])
                    if ci < len(chains):
                        chains[ci]()
                        ci += 1
                    rp = spsum.tile([128, QL], F32, tag="sp", name=f"rp{h}")
                    for half in range(2):
                        nc.tensor.matmul(
                            rp[0:64, half * 512:(half + 1) * 512], ones_s[:, 0:64],
                            rec_h[:, half * 512:(half + 1) * 512],
                            start=True, stop=True)
                    # stage op rows in SBUF (only one DVE input may be PSUM)
                    yrow = ytr_s[hp][hh * 64:(hh + 1) * 64, :]
                    nc.vector.tensor_copy(yrow[:, 0:512], opL[0:64, :])
                    nc.vector.tensor_copy(yrow[:, 512:1024], opR[0:64, :])
                    nc.vector.tensor_tensor(
                        yrow[:, 0:512], yrow[:, 0:512], rp[0:64, 0:512], ALU.mult)
                    nc.vector.tensor_tensor(
                        yrow[:, 512:1024], yrow[:, 512:1024],
                        rp[0:64, 512:1024], ALU.mult)
                while ci < len(chains):
                    chains[ci]()
                    ci += 1

                # this pair's partial out-projection, accumulated into SBUF
                for tc8 in range(8):
                    for n2 in range(2):
                        ps = psum.tile([128, 512], F32, tag="ps",
                                       name=f"o{hp}_{tc8}_{n2}")
                        nc.tensor.matmul(
                            ps[:], ytr_s[hp][:, tc8 * 128:(tc8 + 1) * 128],
                            wp_s[:, hp * D + n2 * 512:hp * D + (n2 + 1) * 512],
                            start=True, stop=True)
                        dst = acc_s[:, tc8 * D + n2 * 512:tc8 * D + (n2 + 1) * 512]
                        if hp == 0:
                            nc.vector.tensor_copy(dst, ps[:])
                        else:
                            nc.vector.tensor_tensor(dst, dst, ps[:], ALU.add)

            # write out
            for tc8 in range(8):
                nc.sync.dma_start(
                    out[tc8 * 128:(tc8 + 1) * 128, :],
                    acc_s[:, tc8 * D:(tc8 + 1) * D])

        if os.environ.get("KV2_PHASES") == "2":
            return


def _to_bf16(a):
    return np.asarray(a, np.float32).astype(ml_dtypes.bfloat16)


def _host_inputs(x, w_attn, b_attn, w_proj, b_proj):
    x = np.asarray(x, np.float32)
    w_attn = np.asarray(w_attn, np.float32)
    b_attn = np.asarray(b_attn, np.float32)
    w_proj = np.asarray(w_proj, np.float32)

    ones1 = np.ones((1, 128), ml_dtypes.bfloat16)
    in_maps = []
    for c in range(8):
        b = c // 4
        qs = (c % 4) // 2
        m = c % 2
        chunks = QSETS[qs]
        hd0 = m * 512                      # head-dim offset of this core's heads

        xb = x[b]                          # [S, D]
        xt = np.ascontiguousarray(xb.T)    # [D, S]
        qcols = np.concatenate(
            [np.arange(ch * GW, (ch + 1) * GW) for ch in chunks])
        xq = np.ascontiguousarray(xb[qcols].T)  # [D, 1024]

        wa = np.concatenate(
            [w_attn[:, hd0:hd0 + 512],                 # Wq slice
             w_attn[:, D + hd0:D + hd0 + 512],         # Wk slice
             w_attn[:, 2 * D + hd0:2 * D + hd0 + 512]  # Wv slice
             ], axis=1)                                # [1024, 1536]
        wp = w_proj[hd0:hd0 + 512, :]                  # [512, 1024]
        bq = b_attn[hd0:hd0 + 512].astype(np.float32) * SCALE
        bk = b_attn[D + hd0:D + hd0 + 512].astype(np.float32)
        bav = b_attn[2 * D + hd0:2 * D + hd0 + 512].astype(np.float32).reshape(1, 512)

        # masks: per group, last 4 slots [128 kv, 4*256]: cols = (slot, q)
        kk = np.arange(128)[:, None]
        ii = np.arange(GW)[None, :]
        diagA = np.where(kk <= ii, 0.0, NEG)           # kv block 2j vs q chunk j
        diagB = np.where(kk <= ii - 128, 0.0, NEG)     # kv block 2j+1
        maskt = np.zeros((128, 4 * 1024), np.float32)
        for g, ch in enumerate(chunks):
            need = 2 * (ch + 1)
            for i4, s in enumerate(range(TG[g] - 4, TG[g])):
                col0 = g * 1024 + i4 * GW
                if s < need - 2:
                    continue                           # fully visible
                elif s == need - 2:
                    maskt[:, col0:col0 + GW] = diagA
                elif s == need - 1:
                    maskt[:, col0:col0 + GW] = diagB
                else:
                    maskt[:, col0:col0 + GW] = NEG     # dummy slot
        identm = np.eye(128, dtype=np.float32)
        in_maps.append({
            "xt": _to_bf16(xt),
            "xq": _to_bf16(xq),
            "wa": _to_bf16(wa),
            "wp": _to_bf16(np.ascontiguousarray(wp)),
            "bq": bq,
            "bk": bk,
            "bav": _to_bf16(bav),
            "maskt": _to_bf16(maskt),
            "ones1": ones1,
            "ident": _to_bf16(identm),
        })
    return in_maps


def _assemble(results, b_proj):
    b_proj = np.asarray(b_proj, np.float32)
    full = np.empty((B, S, D), np.float32)
    for b in range(B):
        for qs in range(2):
            cA = b * 4 + qs * 2
            pA = results[cA]["out"]
            pB = results[cA + 1]["out"]
            tot = pA + pB + b_proj[None, :]
            for g, ch in enumerate(QSETS[qs]):
                full[b, ch * GW:(ch + 1) * GW] = tot[g * GW:(g + 1) * GW]
    return full


def kernel(x, w_attn, b_attn, w_proj, b_proj):
    if "nc" not in _CACHED:
        _CACHED["nc"] = build_nc()
    nc = _CACHED["nc"]
    in_maps = _host_inputs(x, w_attn, b_attn, w_proj, b_proj)
    res = run_bass_kernel_spmd(nc, in_maps, core_ids=list(range(8)))
    return _assemble(res.results, b_proj)


# revision 7
# speedup vs baseline: 1.0285x; 1.0285x over previous
"""Causal self-attention (B=2, S=2048, D=1024, H=16) on 8 TRN2 NeuronCores.

Zero-communication sharding: core c -> (batch b=c//4, query-set qs=(c%4)//2,
head-half m=c%2). The two cores sharing (b, qs) split the 16 heads; each
computes K/V for its 8 heads over the FULL sequence (no replication
anywhere), attention for the query-set's 1024 rows, and a partial output
projection over its 512 head-dims. Host sums the two partials + bias.

Causal handling: per batch, query chunks of 256 rows; query-set 0 gets
chunks [7,5,2,0] (group-ordered), set 1 gets [6,4,3,1]. Attention per head
runs 4 static slot-groups with T=(16,12,8,4) kv-block slots; slot s of
group g multiplies kv block s (128 tokens) against the group's 256 q
columns. The last 4 slots of each group carry a per-core mask (causal
diagonals / -1e9 padding); earlier slots are always fully visible.

V projection runs first, then per head-pair: Q proj, K proj, attention for
its two heads -- the scalar-engine exp backlog of a pair drains while the
PE runs the next pair's projections.

All matmul operands bf16 (full PE rate, all N>=256), f32 PSUM accumulation.
Denominators via an appended ones-column in V (row 64 of the AV psum).
"""

import os
import numpy as np
import ml_dtypes

import concourse.bass as bass
import concourse.mybir as mybir
import concourse.tile as tile
from concourse import bacc
from concourse.bass_utils import run_bass_kernel_spmd

F32 = mybir.dt.float32
BF16 = mybir.dt.bfloat16
AF = mybir.ActivationFunctionType
ALU = mybir.AluOpType

B, S, D, H, HD = 2, 2048, 1024, 16, 64
NKC = D // 128          # 8 contraction chunks of the model dim
NHP = 4                 # head-pairs per core (8 heads)
QL = 1024               # query rows per core
GW = 256                # query-group width
NEG = -1.0e9
SCALE = 1.0 / np.sqrt(HD)
TG = [16, 12, 8, 4]     # kv-block slots per group
SUBT = {g: [2] * (t // 2) for g, t in enumerate(TG)}   # psum sub-tiles of 2 slots

QSETS = [
    [7, 5, 2, 0],   # needs 16,12,6,2  <= TG
    [6, 4, 3, 1],   # needs 14,10,8,4  <= TG
]

_CACHED = {}


def build_nc():
    nc = bacc.Bacc("TRN2", target_bir_lowering=False, debug=False, num_devices=8)

    xt = nc.dram_tensor("xt", [D, S], BF16, kind="ExternalInput").ap()
    xq = nc.dram_tensor("xq", [D, QL], BF16, kind="ExternalInput").ap()
    wa = nc.dram_tensor("wa", [D, 1536], BF16, kind="ExternalInput").ap()
    wp = nc.dram_tensor("wp", [512, D], BF16, kind="ExternalInput").ap()
    bq = nc.dram_tensor("bq", [512], F32, kind="ExternalInput").ap()
    bk = nc.dram_tensor("bk", [512], F32, kind="ExternalInput").ap()
    bav = nc.dram_tensor("bav", [1, 512], BF16, kind="ExternalInput").ap()
    maskt = nc.dram_tensor("maskt", [128, 4 * 1024], BF16, kind="ExternalInput").ap()
    ones1 = nc.dram_tensor("ones1", [1, 128], BF16, kind="ExternalInput").ap()
    ident = nc.dram_tensor("ident", [128, 128], BF16, kind="ExternalInput").ap()
    out = nc.dram_tensor("out", [QL, D], F32, kind="ExternalOutput").ap()

    with tile.TileContext(nc) as tc:
        _body(nc, tc, xt, xq, wa, wp, bq, bk, bav, maskt, ones1, ident, out)
    nc.compile()
    return nc


def _body(nc, tc, xt, xq, wa, wp, bq, bk, bav, maskt, ones1, ident, out):
    with (
        tc.tile_pool(name="const", bufs=1) as const_p,
        tc.tile_pool(name="qt", bufs=1) as qt_p,
        tc.tile_pool(name="kt", bufs=1) as kt_p,
        tc.tile_pool(name="vv", bufs=1) as v_p,
        tc.tile_pool(name="yt", bufs=1) as yt_p,
    ):
        # ---------- constants ----------
        ones_s = const_p.tile([1, 128], BF16)
        nc.sync.dma_start(ones_s[:], ones1)
        ident_s = const_p.tile([128, 128], BF16)
        nc.sync.dma_start(ident_s[:], ident)
        bav_s = const_p.tile([1, 512], BF16)
        nc.sync.dma_start(bav_s[:], bav)
        mask_s = const_p.tile([128, 4 * 1024], BF16)
        bq_s = [const_p.tile([128, 1], F32, tag=f"bq{i}", name=f"bq{i}")
                for i in range(NHP)]
        bk_s = [const_p.tile([128, 1], F32, tag=f"bk{i}", name=f"bk{i}")
                for i in range(NHP)]

        bav_bc = const_p.tile([128, 512], BF16)

        # persistent SBUF: Q^T, K^T, V, y^T
        qt_s = [qt_p.tile([128, QL], BF16, tag=f"qt{hp}", name=f"qt{hp}")
                for hp in range(NHP)]
        kt_s = [kt_p.tile([128, S], BF16, tag=f"kt{hp}", name=f"kt{hp}")
                for hp in range(NHP)]
        # V: 16 kv-blocks x [128, 8 heads * 65]; ones col at slot 64
        v_s = v_p.tile([128, 16 * 520], BF16)
        ytr_s = [yt_p.tile([128, QL], BF16, tag=f"ytr{hp}", name=f"ytr{hp}")
                 for hp in range(NHP)]

        # ---------- phases 1+2 interleaved ----------
        # V proj first (all 16 kv blocks), then per head-pair: Q proj, K proj,
        # and attention for its two heads. The scalar-engine exp backlog of a
        # head-pair drains while the PE runs the next pair's projections.
        TCH = 512
        NTC = S // TCH
        with (
            tc.tile_pool(name="waw", bufs=1) as wa_p,
            tc.tile_pool(name="xqp", bufs=1) as xq_p,
            tc.tile_pool(name="xtc", bufs=1) as xtc_p,
            tc.tile_pool(name="pt", bufs=5) as p_p,
            tc.tile_pool(name="den", bufs=2) as den_p,
            tc.tile_pool(name="psum", bufs=2, space="PSUM") as psum,
            tc.tile_pool(name="spsum", bufs=4, space="PSUM") as spsum,
            tc.tile_pool(name="opsum", bufs=1, space="PSUM") as opsum,
        ):
            t = psum.tile([128, 512], F32, tag="ps", name="bavbc")
            nc.tensor.matmul(t[:], ones_s[:], bav_s[:], start=True, stop=True)
            nc.vector.tensor_copy(bav_bc[:], t[:])

            # batched loads, ordered by first use: Wv + x chunks first (V proj
            # starts after ~2MB lands), then Wq/Wk, xq, biases, masks, wp
            wav_s = wa_p.tile([128, NKC * 512], BF16, tag="wav")
            nc.sync.dma_start(
                wav_s[:].rearrange("p (kc n) -> p kc n", n=512),
                wa[:, 1024:1536].rearrange("(kc p) n -> p kc n", p=128))
            xt_c = [xtc_p.tile([128, NKC * TCH], BF16, tag=f"xt{tcn}",
                               name=f"xtc{tcn}") for tcn in range(NTC)]
            for tcn in range(NTC):
                nc.sync.dma_start(
                    xt_c[tcn][:].rearrange("p (kc n) -> p kc n", n=TCH),
                    xt[:, tcn * TCH:(tcn + 1) * TCH].rearrange(
                        "(kc p) n -> p kc n", p=128))
            waqk_s = wa_p.tile([128, NKC * 1024], BF16, tag="waqk")
            nc.sync.dma_start(
                waqk_s[:].rearrange("p (kc n) -> p kc n", n=1024),
                wa[:, 0:1024].rearrange("(kc p) n -> p kc n", p=128))
            xq_s = xq_p.tile([128, NKC * QL], BF16)
            nc.sync.dma_start(
                xq_s[:].rearrange("p (kc n) -> p kc n", n=QL),
                xq.rearrange("(kc p) n -> p kc n", p=128))
            for hp in range(NHP):
                nc.sync.dma_start(
                    bq_s[hp][:],
                    bq[hp * 128:(hp + 1) * 128].rearrange("(p o) -> p o", o=1))
                nc.sync.dma_start(
                    bk_s[hp][:],
                    bk[hp * 128:(hp + 1) * 128].rearrange("(p o) -> p o", o=1))
            nc.sync.dma_start(mask_s[:], maskt)

            def wa_c(kc, lo, hi):
                if lo >= 1024:
                    return wav_s[:, kc * 512 + lo - 1024:kc * 512 + hi - 1024]
                return waqk_s[:, kc * 1024 + lo:kc * 1024 + hi]

            wp_s = wa_p.tile([128, NHP * D], BF16, tag="wp")
            nc.sync.dma_start(
                wp_s[:].rearrange("p (hp n) -> p hp n", n=D),
                wp.rearrange("(hp p) n -> p hp n", p=128))
            # out-projection accumulator in SBUF: 8 row-chunks x [128, 1024]
            acc_s = yt_p.tile([128, 8 * D], F32, tag="acc")

            # V (all blocks, all 8 heads)
            for tcn in range(NTC):
                for vt in range(TCH // 128):
                    blk = tcn * (TCH // 128) + vt
                    ps = psum.tile([128, 512], F32, tag="ps", name=f"v{blk}")
                    for kc in range(NKC):
                        nc.tensor.matmul(
                            ps[:],
                            xt_c[tcn][:, kc * TCH + vt * 128:kc * TCH + (vt + 1) * 128],
                            wa_c(kc, 1024, 1536),
                            start=(kc == 0), stop=(kc == NKC - 1))
                    v3 = v_s[:, blk * 520:(blk + 1) * 520].rearrange(
                        "p (h e) -> p h e", e=65)
                    nc.vector.tensor_tensor(
                        v3[:, :, 0:64],
                        ps[:].rearrange("p (h e) -> p h e", e=64),
                        bav_bc[:].rearrange("p (h e) -> p h e", e=64),
                        ALU.add)
                    nc.vector.memset(v3[:, :, 64:65], 1.0)

            # projection chain emitters: two Q halves + four K chunks per pair
            def q_chain(hp, half):
                ps = psum.tile([128, 512], F32, tag="ps", name=f"q{hp}_{half}")
                for kc in range(NKC):
                    nc.tensor.matmul(
                        ps[:], wa_c(kc, hp * 128, (hp + 1) * 128),
                        xq_s[:, kc * QL + half * 512:kc * QL + (half + 1) * 512],
                        start=(kc == 0), stop=(kc == NKC - 1))
                nc.vector.tensor_scalar(
                    qt_s[hp][:, half * 512:(half + 1) * 512], ps[:],
                    SCALE, bq_s[hp][:], ALU.mult, ALU.add)

            def k_chain(hp, tcn):
                ps = psum.tile([128, TCH], F32, tag="ps", name=f"k{hp}_{tcn}")
                for kc in range(NKC):
                    nc.tensor.matmul(
                        ps[:], wa_c(kc, 512 + hp * 128, 512 + (hp + 1) * 128),
                        xt_c[tcn][:, kc * TCH:(kc + 1) * TCH],
                        start=(kc == 0), stop=(kc == NKC - 1))
                nc.vector.tensor_scalar(
                    kt_s[hp][:, tcn * TCH:(tcn + 1) * TCH], ps[:],
                    bk_s[hp][:], None, ALU.add)

            def pair_chains(hp):
                out = []
                for half in range(2):
                    out.append(lambda hp=hp, half=half: q_chain(hp, half))
                for tcn in range(NTC):
                    out.append(lambda hp=hp, tcn=tcn: k_chain(hp, tcn))
                return out

            # pair 0's projections up front
            for fn in pair_chains(0):
                fn()

            for hp in range(NHP):
                # chains of the NEXT pair, drip-fed between attention subtiles
                # so the PE has independent work while exp results are pending
                chains = pair_chains(hp + 1) if hp + 1 < NHP else []
                ci = 0
                sub_i = 0

                # attention for this pair's two heads
                for h in (2 * hp, 2 * hp + 1):
                    hh = h % 2
                    opL = opsum.tile([65, 512], F32, tag="opL", name=f"opL{h}")
                    opR = opsum.tile([65, 512], F32, tag="opR", name=f"opR{h}")

                    # flat subtile schedule: (group, first slot, n slots, masked?)
                    sched = []
                    for g in range(4):
                        s0 = 0
                        for si, ns in enumerate(SUBT[g]):
                            sched.append((g, s0, ns, si >= len(SUBT[g]) - 2,
                                          si == len(SUBT[g]) - 2))
                            s0 += ns
                    pend = []

                    def emit_av(item):
                        g, s0, ns, pt = item
                        op_t = opL if g < 2 else opR
                        oc = (g % 2) * GW
                        for sl in range(ns):
                            s = s0 + sl
                            nc.tensor.matmul(
                                op_t[:, oc:oc + GW],
                                v_s[:, s * 520 + h * 65:s * 520 + (h + 1) * 65],
                                pt[:, sl * GW:(sl + 1) * GW],
                                start=(s == 0), stop=(s == TG[g] - 1))

                    for t_i, (g, s0, ns, masked, first_tail) in enumerate(sched):
                        st = spsum.tile([128, ns * GW], F32, tag="sp",
                                        name=f"s{h}_{g}_{s0}")
                        if masked:
                            # open the bank with the mask (I^T @ mask zeroes +
                            # writes it), S matmuls accumulate on top
                            mc = g * 1024 + (0 if first_tail else 512)
                            nc.tensor.matmul(
                                st[:], ident_s[:], mask_s[:, mc:mc + 512],
                                start=True, stop=False)
                        for sl in range(ns):
                            s = s0 + sl
                            nc.tensor.matmul(
                                st[:, sl * GW:(sl + 1) * GW],
                                kt_s[hp][hh * 64:(hh + 1) * 64,
                                         s * 128:(s + 1) * 128],
                                qt_s[hp][hh * 64:(hh + 1) * 64,
                                         g * GW:(g + 1) * GW],
                                start=not masked, stop=(not masked) or sl == ns - 1)
                        pt = p_p.tile([128, ns * GW], BF16, tag="p",
                                      name=f"pt{h}_{g}_{s0}")
                        nc.scalar.activation(pt[:], st[:], AF.Exp)
                        pend.append((g, s0, ns, pt))
                        # drip a projection chain every 6th subtile
                        if ci < len(chains) and sub_i % 6 == 5:
                            chains[ci]()
                            ci += 1
                        sub_i += 1
                        if len(pend) > 2:
                            emit_av(pend.pop(0))
                    while pend:
                        emit_av(pend.pop(0))
                    # normalize: row 64 = denominators
                    den_h = den_p.tile([1, QL], F32, tag="den", name=f"den{h}")
                    nc.vector.tensor_copy(den_h[:, 0:512], opL[64:65, :])
                    nc.vector.tensor_copy(den_h[:, 512:1024], opR[64:65, :])
                    rec_h = den_p.tile([1, QL], BF16, tag="rec", name=f"rec{h}")
                    with nc.allow_low_precision(reason="bf16 reciprocal of denom"):
                        nc.vector.reciprocal(rec_h[:], den_h[:])
                    if ci < len(chains):
                        chains[ci]()
                        ci += 1
                    rp_h = [psum.tile([128, 512], F32, tag="ps",
                                      name=f"rp{h}_{half}") for half in range(2)]
                    for half in range(2):
                        nc.tensor.matmul(
                            rp_h[half][0:64, :], ones_s[:, 0:64],
                            rec_h[:, half * 512:(half + 1) * 512],
                            start=True, stop=True)
                    # stage op rows in SBUF (only one DVE input may be PSUM)
                    yrow = ytr_s[hp][hh * 64:(hh + 1) * 64, :]
                    nc.vector.tensor_copy(yrow[:, 0:512], opL[0:64, :])
                    nc.vector.tensor_copy(yrow[:, 512:1024], opR[0:64, :])
                    nc.vector.tensor_tensor(
                        yrow[:, 0:512], yrow[:, 0:512], rp_h[0][0:64, :], ALU.mult)
                    nc.vector.tensor_tensor(
                        yrow[:, 512:1024], yrow[:, 512:1024],
                        rp_h[1][0:64, :], ALU.mult)
                while ci < len(chains):
                    chains[ci]()
                    ci += 1

                # this pair's partial out-projection, accumulated into SBUF
                for tc8 in range(8):
                    for n2 in range(2):
                        ps = psum.tile([128, 512], F32, tag="ps",
                                       name=f"o{hp}_{tc8}_{n2}")
                        nc.tensor.matmul(
                            ps[:], ytr_s[hp][:, tc8 * 128:(tc8 + 1) * 128],
                            wp_s[:, hp * D + n2 * 512:hp * D + (n2 + 1) * 512],
                            start=True, stop=True)
                        dst = acc_s[:, tc8 * D + n2 * 512:tc8 * D + (n2 + 1) * 512]
                        if hp == 0:
                            nc.vector.tensor_copy(dst, ps[:])
                        else:
                            nc.vector.tensor_tensor(dst, dst, ps[:], ALU.add)

            # write out
            for tc8 in range(8):
                nc.sync.dma_start(
                    out[tc8 * 128:(tc8 + 1) * 128, :],
                    acc_s[:, tc8 * D:(tc8 + 1) * D])

        if os.environ.get("KV2_PHASES") == "2":
            return


def _to_bf16(a):
    return np.asarray(a, np.float32).astype(ml_dtypes.bfloat16)


def _host_inputs(x, w_attn, b_attn, w_proj, b_proj):
    x = np.asarray(x, np.float32)
    w_attn = np.asarray(w_attn, np.float32)
    b_attn = np.asarray(b_attn, np.float32)
    w_proj = np.asarray(w_proj, np.float32)

    ones1 = np.ones((1, 128), ml_dtypes.bfloat16)
    in_maps = []
    for c in range(8):
        b = c // 4
        qs = (c % 4) // 2
        m = c % 2
        chunks = QSETS[qs]
        hd0 = m * 512                      # head-dim offset of this core's heads

        xb = x[b]                          # [S, D]
        xt = np.ascontiguousarray(xb.T)    # [D, S]
        qcols = np.concatenate(
            [np.arange(ch * GW, (ch + 1) * GW) for ch in chunks])
        xq = np.ascontiguousarray(xb[qcols].T)  # [D, 1024]

        wa = np.concatenate(
            [w_attn[:, hd0:hd0 + 512],                 # Wq slice
             w_attn[:, D + hd0:D + hd0 + 512],         # Wk slice
             w_attn[:, 2 * D + hd0:2 * D + hd0 + 512]  # Wv slice
             ], axis=1)                                # [1024, 1536]
        wp = w_proj[hd0:hd0 + 512, :]                  # [512, 1024]
        bq = b_attn[hd0:hd0 + 512].astype(np.float32) * SCALE
        bk = b_attn[D + hd0:D + hd0 + 512].astype(np.float32)
        bav = b_attn[2 * D + hd0:2 * D + hd0 + 512].astype(np.float32).reshape(1, 512)

        # masks: per group, last 4 slots [128 kv, 4*256]: cols = (slot, q)
        kk = np.arange(128)[:, None]
        ii = np.arange(GW)[None, :]
        diagA = np.where(kk <= ii, 0.0, NEG)           # kv block 2j vs q chunk j
        diagB = np.where(kk <= ii - 128, 0.0, NEG)     # kv block 2j+1
        maskt = np.zeros((128, 4 * 1024), np.float32)
        for g, ch in enumerate(chunks):
            need = 2 * (ch + 1)
            for i4, s in enumerate(range(TG[g] - 4, TG[g])):
                col0 = g * 1024 + i4 * GW
                if s < need - 2:
                    continue                           # fully visible
                elif s == need - 2:
                    maskt[:, col0:col0 + GW] = diagA
                elif s == need - 1:
                    maskt[:, col0:col0 + GW] = diagB
                else:
                    maskt[:, col0:col0 + GW] = NEG     # dummy slot
        identm = np.eye(128, dtype=np.float32)
        in_maps.append({
            "xt": _to_bf16(xt),
            "xq": _to_bf16(xq),
            "wa": _to_bf16(wa),
            "wp": _to_bf16(np.ascontiguousarray(wp)),
            "bq": bq,
            "bk": bk,
            "bav": _to_bf16(bav),
            "maskt": _to_bf16(maskt),
            "ones1": ones1,
            "ident": _to_bf16(identm),
        })
    return in_maps


def _assemble(results, b_proj):
    b_proj = np.asarray(b_proj, np.float32)
    full = np.empty((B, S, D), np.float32)
    for b in range(B):
        for qs in range(2):
            cA = b * 4 + qs * 2
            pA = results[cA]["out"]
            pB = results[cA + 1]["out"]
            tot = pA + pB + b_proj[None, :]
            for g, ch in enumerate(QSETS[qs]):
                full[b, ch * GW:(ch + 1) * GW] = tot[g * GW:(g + 1) * GW]
    return full


def kernel(x, w_attn, b_attn, w_proj, b_proj):
    if "nc" not in _CACHED:
        _CACHED["nc"] = build_nc()
    nc = _CACHED["nc"]
    in_maps = _host_inputs(x, w_attn, b_attn, w_proj, b_proj)
    res = run_bass_kernel_spmd(nc, in_maps, core_ids=list(range(8)))
    return _assemble(res.results, b_proj)


# revision 8
# speedup vs baseline: 1.1265x; 1.0952x over previous
"""Causal self-attention (B=2, S=2048, D=1024, H=16) on 8 TRN2 NeuronCores.

Zero-communication sharding: core c -> (batch b=c//4, query-set qs=(c%4)//2,
head-half m=c%2). The two cores sharing (b, qs) split the 16 heads; each
computes K/V for its 8 heads over the FULL sequence (no replication
anywhere), attention for the query-set's 1024 rows, and a partial output
projection over its 512 head-dims. Host sums the two partials + bias.

Causal handling: per batch, query chunks of 256 rows; query-set 0 gets
chunks [7,5,2,0] (group-ordered), set 1 gets [6,4,3,1]. Attention per head
runs 4 static slot-groups with T=(16,12,8,4) kv-block slots; slot s of
group g multiplies kv block s (128 tokens) against the group's 256 q
columns. The last 4 slots of each group carry a per-core mask (causal
diagonals / -1e9 padding); earlier slots are always fully visible.

V projection runs first, then per head-pair: Q proj, K proj, attention for
its two heads -- the scalar-engine exp backlog of a pair drains while the
PE runs the next pair's projections.

All matmul operands bf16 (full PE rate, all N>=256), f32 PSUM accumulation.
Denominators via an appended ones-column in V (row 64 of the AV psum).
"""

import os
import numpy as np
import ml_dtypes

import concourse.bass as bass
import concourse.mybir as mybir
import concourse.tile as tile
from concourse import bacc
from concourse.bass_utils import run_bass_kernel_spmd

F32 = mybir.dt.float32
BF16 = mybir.dt.bfloat16
AF = mybir.ActivationFunctionType
ALU = mybir.AluOpType

B, S, D, H, HD = 2, 2048, 1024, 16, 64
NKC = D // 128          # 8 contraction chunks of the model dim
NHP = 4                 # head-pairs per core (8 heads)
QL = 1024               # query rows per core
GW = 256                # query-group width
NEG = -1.0e9
SCALE = 1.0 / np.sqrt(HD)
TG = [16, 12, 8, 4]     # kv-block slots per group
SUBT = {g: [2] * (t // 2) for g, t in enumerate(TG)}   # psum sub-tiles of 2 slots

QSETS = [
    [7, 5, 2, 0],   # needs 16,12,6,2  <= TG
    [6, 4, 3, 1],   # needs 14,10,8,4  <= TG
]

_CACHED = {}


def build_nc():
    nc = bacc.Bacc("TRN2", target_bir_lowering=False, debug=False, num_devices=8)

    xt = nc.dram_tensor("xt", [D, S], BF16, kind="ExternalInput").ap()
    xq = nc.dram_tensor("xq", [D, QL], BF16, kind="ExternalInput").ap()
    wa = nc.dram_tensor("wa", [D, 1536], BF16, kind="ExternalInput").ap()
    wp = nc.dram_tensor("wp", [512, D], BF16, kind="ExternalInput").ap()
    bq = nc.dram_tensor("bq", [512], F32, kind="ExternalInput").ap()
    bk = nc.dram_tensor("bk", [512], F32, kind="ExternalInput").ap()
    bav = nc.dram_tensor("bav", [1, 512], BF16, kind="ExternalInput").ap()
    maskt = nc.dram_tensor("maskt", [128, 4 * 1024], BF16, kind="ExternalInput").ap()
    ones1 = nc.dram_tensor("ones1", [1, 128], BF16, kind="ExternalInput").ap()
    ident = nc.dram_tensor("ident", [128, 128], BF16, kind="ExternalInput").ap()
    out = nc.dram_tensor("out", [QL, D], F32, kind="ExternalOutput").ap()

    with tile.TileContext(nc) as tc:
        _body(nc, tc, xt, xq, wa, wp, bq, bk, bav, maskt, ones1, ident, out)
    nc.compile()
    return nc


def _body(nc, tc, xt, xq, wa, wp, bq, bk, bav, maskt, ones1, ident, out):
    with (
        tc.tile_pool(name="const", bufs=1) as const_p,
        tc.tile_pool(name="qt", bufs=1) as qt_p,
        tc.tile_pool(name="kt", bufs=1) as kt_p,
        tc.tile_pool(name="vv", bufs=1) as v_p,
        tc.tile_pool(name="yt", bufs=1) as yt_p,
    ):
        # ---------- constants ----------
        ones_s = const_p.tile([1, 128], BF16)
        nc.sync.dma_start(ones_s[:], ones1)
        ident_s = const_p.tile([128, 128], BF16)
        nc.sync.dma_start(ident_s[:], ident)
        bav_s = const_p.tile([1, 512], BF16)
        nc.sync.dma_start(bav_s[:], bav)
        mask_s = const_p.tile([128, 4 * 1024], BF16)
        bq_s = [const_p.tile([128, 1], F32, tag=f"bq{i}", name=f"bq{i}")
                for i in range(NHP)]
        bk_s = [const_p.tile([128, 1], F32, tag=f"bk{i}", name=f"bk{i}")
                for i in range(NHP)]

        bav_bc = const_p.tile([128, 512], BF16)

        # persistent SBUF: Q^T, K^T, V, y^T
        qt_s = [qt_p.tile([128, QL], BF16, tag=f"qt{hp}", name=f"qt{hp}")
                for hp in range(NHP)]
        kt_s = [kt_p.tile([128, S], BF16, tag=f"kt{hp}", name=f"kt{hp}")
                for hp in range(NHP)]
        # V: 16 kv-blocks x [128, 8 heads * 65]; ones col at slot 64
        v_s = v_p.tile([128, 16 * 520], BF16)
        ytr_s = [yt_p.tile([128, QL], BF16, tag=f"ytr{hp}", name=f"ytr{hp}")
                 for hp in range(NHP)]

        # ---------- phases 1+2 interleaved ----------
        # V proj first (all 16 kv blocks), then per head-pair: Q proj, K proj,
        # and attention for its two heads. The scalar-engine exp backlog of a
        # head-pair drains while the PE runs the next pair's projections.
        TCH = 512
        NTC = S // TCH
        with (
            tc.tile_pool(name="waw", bufs=1) as wa_p,
            tc.tile_pool(name="xqp", bufs=1) as xq_p,
            tc.tile_pool(name="xtc", bufs=1) as xtc_p,
            tc.tile_pool(name="pt", bufs=5) as p_p,
            tc.tile_pool(name="den", bufs=2) as den_p,
            tc.tile_pool(name="psum", bufs=2, space="PSUM") as psum,
            tc.tile_pool(name="spsum", bufs=4, space="PSUM") as spsum,
            tc.tile_pool(name="opsum", bufs=1, space="PSUM") as opsum,
        ):
            t = psum.tile([128, 512], F32, tag="ps", name="bavbc")
            nc.tensor.matmul(t[:], ones_s[:], bav_s[:], start=True, stop=True)
            nc.vector.tensor_copy(bav_bc[:], t[:])

            # batched loads, ordered by first use: Wv + x chunks first (V proj
            # starts after ~2MB lands), then Wq/Wk, xq, biases, masks, wp
            wav_s = wa_p.tile([128, NKC * 512], BF16, tag="wav")
            nc.sync.dma_start(
                wav_s[:].rearrange("p (kc n) -> p kc n", n=512),
                wa[:, 1024:1536].rearrange("(kc p) n -> p kc n", p=128))
            xt_c = [xtc_p.tile([128, NKC * TCH], BF16, tag=f"xt{tcn}",
                               name=f"xtc{tcn}") for tcn in range(NTC)]
            for tcn in range(NTC):
                nc.sync.dma_start(
                    xt_c[tcn][:].rearrange("p (kc n) -> p kc n", n=TCH),
                    xt[:, tcn * TCH:(tcn + 1) * TCH].rearrange(
                        "(kc p) n -> p kc n", p=128))
            waqk_s = wa_p.tile([128, NKC * 1024], BF16, tag="waqk")
            nc.sync.dma_start(
                waqk_s[:].rearrange("p (kc n) -> p kc n", n=1024),
                wa[:, 0:1024].rearrange("(kc p) n -> p kc n", p=128))
            xq_s = xq_p.tile([128, NKC * QL], BF16)
            nc.sync.dma_start(
                xq_s[:].rearrange("p (kc n) -> p kc n", n=QL),
                xq.rearrange("(kc p) n -> p kc n", p=128))
            for hp in range(NHP):
                nc.sync.dma_start(
                    bq_s[hp][:],
                    bq[hp * 128:(hp + 1) * 128].rearrange("(p o) -> p o", o=1))
                nc.sync.dma_start(
                    bk_s[hp][:],
                    bk[hp * 128:(hp + 1) * 128].rearrange("(p o) -> p o", o=1))
            nc.sync.dma_start(mask_s[:], maskt)

            def wa_c(kc, lo, hi):
                if lo >= 1024:
                    return wav_s[:, kc * 512 + lo - 1024:kc * 512 + hi - 1024]
                return waqk_s[:, kc * 1024 + lo:kc * 1024 + hi]

            wp_s = wa_p.tile([128, NHP * D], BF16, tag="wp")
            nc.sync.dma_start(
                wp_s[:].rearrange("p (hp n) -> p hp n", n=D),
                wp.rearrange("(hp p) n -> p hp n", p=128))
            # out-projection accumulator in SBUF: 8 row-chunks x [128, 1024]
            acc_s = yt_p.tile([128, 8 * D], F32, tag="acc")

            # V (all blocks, all 8 heads)
            for tcn in range(NTC):
                for vt in range(TCH // 128):
                    blk = tcn * (TCH // 128) + vt
                    ps = psum.tile([128, 512], F32, tag="ps", name=f"v{blk}")
                    for kc in range(NKC):
                        nc.tensor.matmul(
                            ps[:],
                            xt_c[tcn][:, kc * TCH + vt * 128:kc * TCH + (vt + 1) * 128],
                            wa_c(kc, 1024, 1536),
                            start=(kc == 0), stop=(kc == NKC - 1))
                    v3 = v_s[:, blk * 520:(blk + 1) * 520].rearrange(
                        "p (h e) -> p h e", e=65)
                    nc.vector.tensor_tensor(
                        v3[:, :, 0:64],
                        ps[:].rearrange("p (h e) -> p h e", e=64),
                        bav_bc[:].rearrange("p (h e) -> p h e", e=64),
                        ALU.add)
                    nc.vector.memset(v3[:, :, 64:65], 1.0)

            # projection chain emitters: two Q halves + four K chunks per pair
            def q_chain(hp, half):
                ps = psum.tile([128, 512], F32, tag="ps", name=f"q{hp}_{half}")
                for kc in range(NKC):
                    nc.tensor.matmul(
                        ps[:], wa_c(kc, hp * 128, (hp + 1) * 128),
                        xq_s[:, kc * QL + half * 512:kc * QL + (half + 1) * 512],
                        start=(kc == 0), stop=(kc == NKC - 1))
                nc.vector.tensor_scalar(
                    qt_s[hp][:, half * 512:(half + 1) * 512], ps[:],
                    SCALE, bq_s[hp][:], ALU.mult, ALU.add)

            def k_chain(hp, tcn):
                ps = psum.tile([128, TCH], F32, tag="ps", name=f"k{hp}_{tcn}")
                for kc in range(NKC):
                    nc.tensor.matmul(
                        ps[:], wa_c(kc, 512 + hp * 128, 512 + (hp + 1) * 128),
                        xt_c[tcn][:, kc * TCH:(kc + 1) * TCH],
                        start=(kc == 0), stop=(kc == NKC - 1))
                nc.vector.tensor_scalar(
                    kt_s[hp][:, tcn * TCH:(tcn + 1) * TCH], ps[:],
                    bk_s[hp][:], None, ALU.add)

            def pair_chains(hp):
                out = []
                for half in range(2):
                    out.append(lambda hp=hp, half=half: q_chain(hp, half))
                for tcn in range(NTC):
                    out.append(lambda hp=hp, tcn=tcn: k_chain(hp, tcn))
                return out

            # pair 0's projections up front
            for fn in pair_chains(0):
                fn()

            for hp in range(NHP):
                # chains of the NEXT pair, drip-fed between attention subtiles
                # so the PE has independent work while exp results are pending
                chains = pair_chains(hp + 1) if hp + 1 < NHP else []
                ci = 0
                sub_i = 0

                # attention for this pair's two heads
                for h in (2 * hp, 2 * hp + 1):
                    hh = h % 2
                    opL = opsum.tile([65, 512], F32, tag="opL", name=f"opL{h}")
                    opR = opsum.tile([65, 512], F32, tag="opR", name=f"opR{h}")

                    # flat subtile schedule: (group, first slot, n slots, masked?)
                    sched = []
                    for g in range(4):
                        s0 = 0
                        for si, ns in enumerate(SUBT[g]):
                            sched.append((g, s0, ns, si >= len(SUBT[g]) - 2,
                                          si == len(SUBT[g]) - 2))
                            s0 += ns
                    pend = []

                    def emit_av(item):
                        g, s0, ns, pt = item
                        op_t = opL if g < 2 else opR
                        oc = (g % 2) * GW
                        for sl in range(ns):
                            s = s0 + sl
                            nc.tensor.matmul(
                                op_t[:, oc:oc + GW],
                                v_s[:, s * 520 + h * 65:s * 520 + (h + 1) * 65],
                                pt[:, sl * GW:(sl + 1) * GW],
                                start=(s == 0), stop=(s == TG[g] - 1))

                    for t_i, (g, s0, ns, masked, first_tail) in enumerate(sched):
                        st = spsum.tile([128, ns * GW], F32, tag="sp",
                                        name=f"s{h}_{g}_{s0}")
                        if masked:
                            # open the bank with the mask (I^T @ mask zeroes +
                            # writes it), S matmuls accumulate on top
                            mc = g * 1024 + (0 if first_tail else 512)
                            nc.tensor.matmul(
                                st[:], ident_s[:], mask_s[:, mc:mc + 512],
                                start=True, stop=False)
                        for sl in range(ns):
                            s = s0 + sl
                            nc.tensor.matmul(
                                st[:, sl * GW:(sl + 1) * GW],
                                kt_s[hp][hh * 64:(hh + 1) * 64,
                                         s * 128:(s + 1) * 128],
                                qt_s[hp][hh * 64:(hh + 1) * 64,
                                         g * GW:(g + 1) * GW],
                                start=not masked, stop=(not masked) or sl == ns - 1)
                        pt = p_p.tile([128, ns * GW], BF16, tag="p",
                                      name=f"pt{h}_{g}_{s0}")
                        nc.scalar.activation(pt[:], st[:], AF.Exp)
                        pend.append((g, s0, ns, pt))
                        # drip a projection chain every 6th subtile
                        if ci < len(chains) and sub_i % 6 == 5:
                            chains[ci]()
                            ci += 1
                        sub_i += 1
                        if len(pend) > 3:
                            emit_av(pend.pop(0))
                    while pend:
                        emit_av(pend.pop(0))
                    # normalize: row 64 = denominators
                    den_h = den_p.tile([1, QL], F32, tag="den", name=f"den{h}")
                    nc.vector.tensor_copy(den_h[:, 0:512], opL[64:65, :])
                    nc.vector.tensor_copy(den_h[:, 512:1024], opR[64:65, :])
                    rec_h = den_p.tile([1, QL], BF16, tag="rec", name=f"rec{h}")
                    with nc.allow_low_precision(reason="bf16 reciprocal of denom"):
                        nc.vector.reciprocal(rec_h[:], den_h[:])
                    if ci < len(chains):
                        chains[ci]()
                        ci += 1
                    rp_h = [psum.tile([128, 512], F32, tag="ps",
                                      name=f"rp{h}_{half}") for half in range(2)]
                    for half in range(2):
                        nc.tensor.matmul(
                            rp_h[half][0:64, :], ones_s[:, 0:64],
                            rec_h[:, half * 512:(half + 1) * 512],
                            start=True, stop=True)
                    # stage op rows in SBUF (only one DVE input may be PSUM)
                    yrow = ytr_s[hp][hh * 64:(hh + 1) * 64, :]
                    nc.vector.tensor_copy(yrow[:, 0:512], opL[0:64, :])
                    nc.vector.tensor_copy(yrow[:, 512:1024], opR[0:64, :])
                    nc.vector.tensor_tensor(
                        yrow[:, 0:512], yrow[:, 0:512], rp_h[0][0:64, :], ALU.mult)
                    nc.vector.tensor_tensor(
                        yrow[:, 512:1024], yrow[:, 512:1024],
                        rp_h[1][0:64, :], ALU.mult)
                while ci < len(chains):
                    chains[ci]()
                    ci += 1

                # this pair's partial out-projection, accumulated into SBUF
                for tc8 in range(8):
                    for n2 in range(2):
                        ps = psum.tile([128, 512], F32, tag="ps",
                                       name=f"o{hp}_{tc8}_{n2}")
                        nc.tensor.matmul(
                            ps[:], ytr_s[hp][:, tc8 * 128:(tc8 + 1) * 128],
                            wp_s[:, hp * D + n2 * 512:hp * D + (n2 + 1) * 512],
                            start=True, stop=True)
                        dst = acc_s[:, tc8 * D + n2 * 512:tc8 * D + (n2 + 1) * 512]
                        if hp == 0:
                            nc.vector.tensor_copy(dst, ps[:])
                        else:
                            nc.vector.tensor_tensor(dst, dst, ps[:], ALU.add)

            # write out
            for tc8 in range(8):
                nc.sync.dma_start(
                    out[tc8 * 128:(tc8 + 1) * 128, :],
                    acc_s[:, tc8 * D:(tc8 + 1) * D])

        if os.environ.get("KV2_PHASES") == "2":
            return


def _to_bf16(a):
    return np.asarray(a, np.float32).astype(ml_dtypes.bfloat16)


def _host_inputs(x, w_attn, b_attn, w_proj, b_proj):
    x = np.asarray(x, np.float32)
    w_attn = np.asarray(w_attn, np.float32)
    b_attn = np.asarray(b_attn, np.float32)
    w_proj = np.asarray(w_proj, np.float32)

    ones1 = np.ones((1, 128), ml_dtypes.bfloat16)
    in_maps = []
    for c in range(8):
        b = c // 4
        qs = (c % 4) // 2
        m = c % 2
        chunks = QSETS[qs]
        hd0 = m * 512                      # head-dim offset of this core's heads

        xb = x[b]                          # [S, D]
        xt = np.ascontiguousarray(xb.T)    # [D, S]
        qcols = np.concatenate(
            [np.arange(ch * GW, (ch + 1) * GW) for ch in chunks])
        xq = np.ascontiguousarray(xb[qcols].T)  # [D, 1024]

        wa = np.concatenate(
            [w_attn[:, hd0:hd0 + 512],                 # Wq slice
             w_attn[:, D + hd0:D + hd0 + 512],         # Wk slice
             w_attn[:, 2 * D + hd0:2 * D + hd0 + 512]  # Wv slice
             ], axis=1)                                # [1024, 1536]
        wp = w_proj[hd0:hd0 + 512, :]                  # [512, 1024]
        bq = b_attn[hd0:hd0 + 512].astype(np.float32) * SCALE
        bk = b_attn[D + hd0:D + hd0 + 512].astype(np.float32)
        bav = b_attn[2 * D + hd0:2 * D + hd0 + 512].astype(np.float32).reshape(1, 512)

        # masks: per group, last 4 slots [128 kv, 4*256]: cols = (slot, q)
        kk = np.arange(128)[:, None]
        ii = np.arange(GW)[None, :]
        diagA = np.where(kk <= ii, 0.0, NEG)           # kv block 2j vs q chunk j
        diagB = np.where(kk <= ii - 128, 0.0, NEG)     # kv block 2j+1
        maskt = np.zeros((128, 4 * 1024), np.float32)
        for g, ch in enumerate(chunks):
            need = 2 * (ch + 1)
            for i4, s in enumerate(range(TG[g] - 4, TG[g])):
                col0 = g * 1024 + i4 * GW
                if s < need - 2:
                    continue                           # fully visible
                elif s == need - 2:
                    maskt[:, col0:col0 + GW] = diagA
                elif s == need - 1:
                    maskt[:, col0:col0 + GW] = diagB
                else:
                    maskt[:, col0:col0 + GW] = NEG     # dummy slot
        identm = np.eye(128, dtype=np.float32)
        in_maps.append({
            "xt": _to_bf16(xt),
            "xq": _to_bf16(xq),
            "wa": _to_bf16(wa),
            "wp": _to_bf16(np.ascontiguousarray(wp)),
            "bq": bq,
            "bk": bk,
            "bav": _to_bf16(bav),
            "maskt": _to_bf16(maskt),
            "ones1": ones1,
            "ident": _to_bf16(identm),
        })
    return in_maps


def _assemble(results, b_proj):
    b_proj = np.asarray(b_proj, np.float32)
    full = np.empty((B, S, D), np.float32)
    for b in range(B):
        for qs in range(2):
            cA = b * 4 + qs * 2
            pA = results[cA]["out"]
            pB = results[cA + 1]["out"]
            tot = pA + pB + b_proj[None, :]
            for g, ch in enumerate(QSETS[qs]):
                full[b, ch * GW:(ch + 1) * GW] = tot[g * GW:(g + 1) * GW]
    return full


def kernel(x, w_attn, b_attn, w_proj, b_proj):
    if "nc" not in _CACHED:
        _CACHED["nc"] = build_nc()
    nc = _CACHED["nc"]
    in_maps = _host_inputs(x, w_attn, b_attn, w_proj, b_proj)
    res = run_bass_kernel_spmd(nc, in_maps, core_ids=list(range(8)))
    return _assemble(res.results, b_proj)


# revision 9
# speedup vs baseline: 1.1721x; 1.0404x over previous
"""Causal self-attention (B=2, S=2048, D=1024, H=16) on 8 TRN2 NeuronCores.

Zero-communication sharding: core c -> (batch b=c//4, query-set qs=(c%4)//2,
head-half m=c%2). The two cores sharing (b, qs) split the 16 heads; each
computes K/V for its 8 heads over the FULL sequence (no replication
anywhere), attention for the query-set's 1024 rows, and a partial output
projection over its 512 head-dims. Host sums the two partials + bias.

Causal handling: per batch, query chunks of 256 rows; query-set 0 gets
chunks [7,5,2,0] (group-ordered), set 1 gets [6,4,3,1]. Attention per head
runs 4 static slot-groups with T=(16,12,8,4) kv-block slots; slot s of
group g multiplies kv block s (128 tokens) against the group's 256 q
columns. The last 4 slots of each group carry a per-core mask (causal
diagonals / -1e9 padding); earlier slots are always fully visible.

V projection runs first, then per head-pair: Q proj, K proj, attention for
its two heads -- the scalar-engine exp backlog of a pair drains while the
PE runs the next pair's projections.

All matmul operands bf16 (full PE rate, all N>=256), f32 PSUM accumulation.
Denominators via an appended ones-column in V (row 64 of the AV psum).
"""

import os
import numpy as np
import ml_dtypes

import concourse.bass as bass
import concourse.mybir as mybir
import concourse.tile as tile
from concourse import bacc
from concourse.bass_utils import run_bass_kernel_spmd

F32 = mybir.dt.float32
BF16 = mybir.dt.bfloat16
AF = mybir.ActivationFunctionType
ALU = mybir.AluOpType

B, S, D, H, HD = 2, 2048, 1024, 16, 64
NKC = D // 128          # 8 contraction chunks of the model dim
NHP = 4                 # head-pairs per core (8 heads)
QL = 1024               # query rows per core
GW = 256                # query-group width
NEG = -1.0e9
SCALE = 1.0 / np.sqrt(HD)
TG = [16, 12, 8, 4]     # kv-block slots per group
SUBT = {g: [2] * (t // 2) for g, t in enumerate(TG)}   # psum sub-tiles of 2 slots

QSETS = [
    [7, 5, 2, 0],   # needs 16,12,6,2  <= TG
    [6, 4, 3, 1],   # needs 14,10,8,4  <= TG
]

_CACHED = {}


def build_nc():
    nc = bacc.Bacc("TRN2", target_bir_lowering=False, debug=False, num_devices=8)

    xt = nc.dram_tensor("xt", [D, S], BF16, kind="ExternalInput").ap()
    xq = nc.dram_tensor("xq", [D, QL], BF16, kind="ExternalInput").ap()
    wa = nc.dram_tensor("wa", [D, 1536], BF16, kind="ExternalInput").ap()
    wp = nc.dram_tensor("wp", [512, D], BF16, kind="ExternalInput").ap()
    bq = nc.dram_tensor("bq", [512], F32, kind="ExternalInput").ap()
    bk = nc.dram_tensor("bk", [512], F32, kind="ExternalInput").ap()
    bav = nc.dram_tensor("bav", [1, 512], BF16, kind="ExternalInput").ap()
    maskt = nc.dram_tensor("maskt", [128, 4 * 1024], BF16, kind="ExternalInput").ap()
    ones1 = nc.dram_tensor("ones1", [1, 128], BF16, kind="ExternalInput").ap()
    ident = nc.dram_tensor("ident", [128, 128], BF16, kind="ExternalInput").ap()
    out = nc.dram_tensor("out", [QL, D], F32, kind="ExternalOutput").ap()

    with tile.TileContext(nc) as tc:
        _body(nc, tc, xt, xq, wa, wp, bq, bk, bav, maskt, ones1, ident, out)
    nc.compile()
    return nc


def _body(nc, tc, xt, xq, wa, wp, bq, bk, bav, maskt, ones1, ident, out):
    with (
        tc.tile_pool(name="const", bufs=1) as const_p,
        tc.tile_pool(name="qt", bufs=1) as qt_p,
        tc.tile_pool(name="kt", bufs=1) as kt_p,
        tc.tile_pool(name="vv", bufs=1) as v_p,
        tc.tile_pool(name="yt", bufs=1) as yt_p,
    ):
        # ---------- constants ----------
        ones_s = const_p.tile([1, 128], BF16)
        nc.sync.dma_start(ones_s[:], ones1)
        ident_s = const_p.tile([128, 128], BF16)
        nc.sync.dma_start(ident_s[:], ident)
        bav_s = const_p.tile([1, 512], BF16)
        nc.sync.dma_start(bav_s[:], bav)
        mask_s = const_p.tile([128, 4 * 1024], BF16)
        bq_s = [const_p.tile([128, 1], F32, tag=f"bq{i}", name=f"bq{i}")
                for i in range(NHP)]
        bk_s = [const_p.tile([128, 1], F32, tag=f"bk{i}", name=f"bk{i}")
                for i in range(NHP)]

        bav_bc = const_p.tile([128, 512], BF16)

        # persistent SBUF: Q^T, K^T, V, y^T
        qt_s = [qt_p.tile([128, QL], BF16, tag=f"qt{hp}", name=f"qt{hp}")
                for hp in range(NHP)]
        kt_s = [kt_p.tile([128, S], BF16, tag=f"kt{hp}", name=f"kt{hp}")
                for hp in range(NHP)]
        # V: 16 kv-blocks x [128, 8 heads * 65]; ones col at slot 64
        v_s = v_p.tile([128, 16 * 520], BF16)
        ytr_s = [yt_p.tile([128, QL], BF16, tag=f"ytr{hp}", name=f"ytr{hp}")
                 for hp in range(NHP)]

        # ---------- phases 1+2 interleaved ----------
        # V proj first (all 16 kv blocks), then per head-pair: Q proj, K proj,
        # and attention for its two heads. The scalar-engine exp backlog of a
        # head-pair drains while the PE runs the next pair's projections.
        TCH = 512
        NTC = S // TCH
        with (
            tc.tile_pool(name="waw", bufs=1) as wa_p,
            tc.tile_pool(name="xqp", bufs=1) as xq_p,
            tc.tile_pool(name="xtc", bufs=1) as xtc_p,
            tc.tile_pool(name="pt", bufs=5) as p_p,
            tc.tile_pool(name="den", bufs=2) as den_p,
            tc.tile_pool(name="psum", bufs=2, space="PSUM") as psum,
            tc.tile_pool(name="spsum", bufs=4, space="PSUM") as spsum,
            tc.tile_pool(name="opsum", bufs=1, space="PSUM") as opsum,
        ):
            t = psum.tile([128, 512], F32, tag="ps", name="bavbc")
            nc.tensor.matmul(t[:], ones_s[:], bav_s[:], start=True, stop=True)
            nc.vector.tensor_copy(bav_bc[:], t[:])

            # batched loads, ordered by first use: Wv + x chunks first (V proj
            # starts after ~2MB lands), then Wq/Wk, xq, biases, masks, wp
            wav_s = wa_p.tile([128, NKC * 512], BF16, tag="wav")
            nc.sync.dma_start(
                wav_s[:].rearrange("p (kc n) -> p kc n", n=512),
                wa[:, 1024:1536].rearrange("(kc p) n -> p kc n", p=128))
            xt_c = [xtc_p.tile([128, NKC * TCH], BF16, tag=f"xt{tcn}",
                               name=f"xtc{tcn}") for tcn in range(NTC)]
            for tcn in range(NTC):
                nc.sync.dma_start(
                    xt_c[tcn][:].rearrange("p (kc n) -> p kc n", n=TCH),
                    xt[:, tcn * TCH:(tcn + 1) * TCH].rearrange(
                        "(kc p) n -> p kc n", p=128))
            waqk_s = wa_p.tile([128, NKC * 1024], BF16, tag="waqk")
            nc.sync.dma_start(
                waqk_s[:].rearrange("p (kc n) -> p kc n", n=1024),
                wa[:, 0:1024].rearrange("(kc p) n -> p kc n", p=128))
            xq_s = xq_p.tile([128, NKC * QL], BF16)
            nc.sync.dma_start(
                xq_s[:].rearrange("p (kc n) -> p kc n", n=QL),
                xq.rearrange("(kc p) n -> p kc n", p=128))
            for hp in range(NHP):
                nc.sync.dma_start(
                    bq_s[hp][:],
                    bq[hp * 128:(hp + 1) * 128].rearrange("(p o) -> p o", o=1))
                nc.sync.dma_start(
                    bk_s[hp][:],
                    bk[hp * 128:(hp + 1) * 128].rearrange("(p o) -> p o", o=1))
            nc.sync.dma_start(mask_s[:], maskt)

            def wa_c(kc, lo, hi):
                if lo >= 1024:
                    return wav_s[:, kc * 512 + lo - 1024:kc * 512 + hi - 1024]
                return waqk_s[:, kc * 1024 + lo:kc * 1024 + hi]

            wp_s = wa_p.tile([128, NHP * D], BF16, tag="wp")
            nc.sync.dma_start(
                wp_s[:].rearrange("p (hp n) -> p hp n", n=D),
                wp.rearrange("(hp p) n -> p hp n", p=128))
            # out-projection accumulator in SBUF: 8 row-chunks x [128, 1024]
            acc_s = yt_p.tile([128, 8 * D], F32, tag="acc")

            # V (all blocks, all 8 heads)
            for tcn in range(NTC):
                for vt in range(TCH // 128):
                    blk = tcn * (TCH // 128) + vt
                    ps = psum.tile([128, 512], F32, tag="ps", name=f"v{blk}")
                    for kc in range(NKC):
                        nc.tensor.matmul(
                            ps[:],
                            xt_c[tcn][:, kc * TCH + vt * 128:kc * TCH + (vt + 1) * 128],
                            wa_c(kc, 1024, 1536),
                            start=(kc == 0), stop=(kc == NKC - 1))
                    v3 = v_s[:, blk * 520:(blk + 1) * 520].rearrange(
                        "p (h e) -> p h e", e=65)
                    nc.vector.tensor_tensor(
                        v3[:, :, 0:64],
                        ps[:].rearrange("p (h e) -> p h e", e=64),
                        bav_bc[:].rearrange("p (h e) -> p h e", e=64),
                        ALU.add)
                    nc.vector.memset(v3[:, :, 64:65], 1.0)

            # projection chain emitters: two Q halves + four K chunks per pair
            def q_chain(hp, half):
                ps = psum.tile([128, 512], F32, tag="ps", name=f"q{hp}_{half}")
                for kc in range(NKC):
                    nc.tensor.matmul(
                        ps[:], wa_c(kc, hp * 128, (hp + 1) * 128),
                        xq_s[:, kc * QL + half * 512:kc * QL + (half + 1) * 512],
                        start=(kc == 0), stop=(kc == NKC - 1))
                nc.vector.tensor_scalar(
                    qt_s[hp][:, half * 512:(half + 1) * 512], ps[:],
                    SCALE, bq_s[hp][:], ALU.mult, ALU.add)

            def k_chain(hp, tcn):
                ps = psum.tile([128, TCH], F32, tag="ps", name=f"k{hp}_{tcn}")
                for kc in range(NKC):
                    nc.tensor.matmul(
                        ps[:], wa_c(kc, 512 + hp * 128, 512 + (hp + 1) * 128),
                        xt_c[tcn][:, kc * TCH:(kc + 1) * TCH],
                        start=(kc == 0), stop=(kc == NKC - 1))
                nc.vector.tensor_scalar(
                    kt_s[hp][:, tcn * TCH:(tcn + 1) * TCH], ps[:],
                    bk_s[hp][:], None, ALU.add)

            def pair_chains(hp):
                out = []
                for half in range(2):
                    out.append(lambda hp=hp, half=half: q_chain(hp, half))
                for tcn in range(NTC):
                    out.append(lambda hp=hp, tcn=tcn: k_chain(hp, tcn))
                return out

            # pair 0's projections up front
            for fn in pair_chains(0):
                fn()

            for hp in range(NHP):
                # chains of the NEXT pair, drip-fed between attention subtiles
                # so the PE has independent work while exp results are pending
                chains = pair_chains(hp + 1) if hp + 1 < NHP else []
                ci = 0
                sub_i = 0

                # attention for this pair's two heads
                for h in (2 * hp, 2 * hp + 1):
                    hh = h % 2
                    opL = opsum.tile([65, 512], F32, tag="opL", name=f"opL{h}")
                    opR = opsum.tile([65, 512], F32, tag="opR", name=f"opR{h}")

                    # flat subtile schedule: (group, first slot, n slots, masked?)
                    sched = []
                    for g in range(4):
                        s0 = 0
                        for si, ns in enumerate(SUBT[g]):
                            sched.append((g, s0, ns, si >= len(SUBT[g]) - 2,
                                          si == len(SUBT[g]) - 2))
                            s0 += ns
                    pend = []

                    def emit_av(item):
                        g, s0, ns, pt = item
                        op_t = opL if g < 2 else opR
                        oc = (g % 2) * GW
                        for sl in range(ns):
                            s = s0 + sl
                            nc.tensor.matmul(
                                op_t[:, oc:oc + GW],
                                v_s[:, s * 520 + h * 65:s * 520 + (h + 1) * 65],
                                pt[:, sl * GW:(sl + 1) * GW],
                                start=(s == 0), stop=(s == TG[g] - 1))

                    for t_i, (g, s0, ns, masked, first_tail) in enumerate(sched):
                        st = spsum.tile([128, ns * GW], F32, tag="sp",
                                        name=f"s{h}_{g}_{s0}")
                        if masked:
                            # open the bank with the mask (I^T @ mask zeroes +
                            # writes it), S matmuls accumulate on top
                            mc = g * 1024 + (0 if first_tail else 512)
                            nc.tensor.matmul(
                                st[:], ident_s[:], mask_s[:, mc:mc + 512],
                                start=True, stop=False)
                        for sl in range(ns):
                            s = s0 + sl
                            nc.tensor.matmul(
                                st[:, sl * GW:(sl + 1) * GW],
                                kt_s[hp][hh * 64:(hh + 1) * 64,
                                         s * 128:(s + 1) * 128],
                                qt_s[hp][hh * 64:(hh + 1) * 64,
                                         g * GW:(g + 1) * GW],
                                start=not masked, stop=(not masked) or sl == ns - 1)
                        pt = p_p.tile([128, ns * GW], BF16, tag="p",
                                      name=f"pt{h}_{g}_{s0}")
                        nc.scalar.activation(pt[:], st[:], AF.Exp)
                        pend.append((g, s0, ns, pt))
                        # drip a projection chain every 6th subtile
                        if ci < len(chains) and sub_i % 6 == 5:
                            chains[ci]()
                            ci += 1
                        sub_i += 1
                        if len(pend) > 3:
                            emit_av(pend.pop(0))
                    while pend:
                        emit_av(pend.pop(0))
                    # normalize: row 64 = denominators
                    den_h = den_p.tile([1, QL], F32, tag="den", name=f"den{h}")
                    nc.vector.tensor_copy(den_h[:, 0:512], opL[64:65, :])
                    nc.vector.tensor_copy(den_h[:, 512:1024], opR[64:65, :])
                    # read out the op accumulators immediately so their psum
                    # banks free up for the next head
                    yrow = ytr_s[hp][hh * 64:(hh + 1) * 64, :]
                    nc.vector.tensor_copy(yrow[:, 0:512], opL[0:64, :])
                    nc.vector.tensor_copy(yrow[:, 512:1024], opR[0:64, :])
                    rec_h = den_p.tile([1, QL], BF16, tag="rec", name=f"rec{h}")
                    with nc.allow_low_precision(reason="bf16 reciprocal of denom"):
                        nc.vector.reciprocal(rec_h[:], den_h[:])
                    if ci < len(chains):
                        chains[ci]()
                        ci += 1
                    rp_h = [psum.tile([128, 512], F32, tag="ps",
                                      name=f"rp{h}_{half}") for half in range(2)]
                    for half in range(2):
                        nc.tensor.matmul(
                            rp_h[half][0:64, :], ones_s[:, 0:64],
                            rec_h[:, half * 512:(half + 1) * 512],
                            start=True, stop=True)
                    nc.vector.tensor_tensor(
                        yrow[:, 0:512], yrow[:, 0:512], rp_h[0][0:64, :], ALU.mult)
                    nc.vector.tensor_tensor(
                        yrow[:, 512:1024], yrow[:, 512:1024],
                        rp_h[1][0:64, :], ALU.mult)
                while ci < len(chains):
                    chains[ci]()
                    ci += 1

                # this pair's partial out-projection, accumulated into SBUF
                for tc8 in range(8):
                    for n2 in range(2):
                        ps = psum.tile([128, 512], F32, tag="ps",
                                       name=f"o{hp}_{tc8}_{n2}")
                        nc.tensor.matmul(
                            ps[:], ytr_s[hp][:, tc8 * 128:(tc8 + 1) * 128],
                            wp_s[:, hp * D + n2 * 512:hp * D + (n2 + 1) * 512],
                            start=True, stop=True)
                        dst = acc_s[:, tc8 * D + n2 * 512:tc8 * D + (n2 + 1) * 512]
                        if hp == 0:
                            nc.vector.tensor_copy(dst, ps[:])
                        else:
                            nc.vector.tensor_tensor(dst, dst, ps[:], ALU.add)

            # write out
            for tc8 in range(8):
                nc.sync.dma_start(
                    out[tc8 * 128:(tc8 + 1) * 128, :],
                    acc_s[:, tc8 * D:(tc8 + 1) * D])

        if os.environ.get("KV2_PHASES") == "2":
            return


def _to_bf16(a):
    return np.asarray(a, np.float32).astype(ml_dtypes.bfloat16)


def _host_inputs(x, w_attn, b_attn, w_proj, b_proj):
    x = np.asarray(x, np.float32)
    w_attn = np.asarray(w_attn, np.float32)
    b_attn = np.asarray(b_attn, np.float32)
    w_proj = np.asarray(w_proj, np.float32)

    ones1 = np.ones((1, 128), ml_dtypes.bfloat16)
    in_maps = []
    for c in range(8):
        b = c // 4
        qs = (c % 4) // 2
        m = c % 2
        chunks = QSETS[qs]
        hd0 = m * 512                      # head-dim offset of this core's heads

        xb = x[b]                          # [S, D]
        xt = np.ascontiguousarray(xb.T)    # [D, S]
        qcols = np.concatenate(
            [np.arange(ch * GW, (ch + 1) * GW) for ch in chunks])
        xq = np.ascontiguousarray(xb[qcols].T)  # [D, 1024]

        wa = np.concatenate(
            [w_attn[:, hd0:hd0 + 512],                 # Wq slice
             w_attn[:, D + hd0:D + hd0 + 512],         # Wk slice
             w_attn[:, 2 * D + hd0:2 * D + hd0 + 512]  # Wv slice
             ], axis=1)                                # [1024, 1536]
        wp = w_proj[hd0:hd0 + 512, :]                  # [512, 1024]
        bq = b_attn[hd0:hd0 + 512].astype(np.float32) * SCALE
        bk = b_attn[D + hd0:D + hd0 + 512].astype(np.float32)
        bav = b_attn[2 * D + hd0:2 * D + hd0 + 512].astype(np.float32).reshape(1, 512)

        # masks: per group, last 4 slots [128 kv, 4*256]: cols = (slot, q)
        kk = np.arange(128)[:, None]
        ii = np.arange(GW)[None, :]
        diagA = np.where(kk <= ii, 0.0, NEG)           # kv block 2j vs q chunk j
        diagB = np.where(kk <= ii - 128, 0.0, NEG)     # kv block 2j+1
        maskt = np.zeros((128, 4 * 1024), np.float32)
        for g, ch in enumerate(chunks):
            need = 2 * (ch + 1)
            for i4, s in enumerate(range(TG[g] - 4, TG[g])):
                col0 = g * 1024 + i4 * GW
                if s < need - 2:
                    continue                           # fully visible
                elif s == need - 2:
                    maskt[:, col0:col0 + GW] = diagA
                elif s == need - 1:
                    maskt[:, col0:col0 + GW] = diagB
                else:
                    maskt[:, col0:col0 + GW] = NEG     # dummy slot
        identm = np.eye(128, dtype=np.float32)
        in_maps.append({
            "xt": _to_bf16(xt),
            "xq": _to_bf16(xq),
            "wa": _to_bf16(wa),
            "wp": _to_bf16(np.ascontiguousarray(wp)),
            "bq": bq,
            "bk": bk,
            "bav": _to_bf16(bav),
            "maskt": _to_bf16(maskt),
            "ones1": ones1,
            "ident": _to_bf16(identm),
        })
    return in_maps


def _assemble(results, b_proj):
    b_proj = np.asarray(b_proj, np.float32)
    full = np.empty((B, S, D), np.float32)
    for b in range(B):
        for qs in range(2):
            cA = b * 4 + qs * 2
            pA = results[cA]["out"]
            pB = results[cA + 1]["out"]
            tot = pA + pB + b_proj[None, :]
            for g, ch in enumerate(QSETS[qs]):
                full[b, ch * GW:(ch + 1) * GW] = tot[g * GW:(g + 1) * GW]
    return full


def kernel(x, w_attn, b_attn, w_proj, b_proj):
    if "nc" not in _CACHED:
        _CACHED["nc"] = build_nc()
    nc = _CACHED["nc"]
    in_maps = _host_inputs(x, w_attn, b_attn, w_proj, b_proj)
    res = run_bass_kernel_spmd(nc, in_maps, core_ids=list(range(8)))
    return _assemble(res.results, b_proj)
